# revision 1
# baseline (speedup 1.0000x reference)
"""Trainium2 Bass kernel for nn_Mlp_13099650253522 (BitNet-ternary dense MLP).

  h = gelu(x @ ter_quant(w1).T + b1);  y = h @ ter_quant(w2).T + b2
  ter_quant(w) = clip(round(w / g), -1, 1) * g,  g = mean(|w|) + 1e-5

Strategy (8 NeuronCores, data-parallel over the 64*197=12608 tokens):
 - Host: transpose x/w1/w2 (layout only), shard tokens 1576/core.
 - Device (per core, identical program):
     * gamma via ACT Abs+accum row sums + GPSIMD partition_all_reduce
     * ternary quant T = Sign(w+g/2)+Sign(w-g/2) in {-2,0,+2}, stored fp8
       (exact); the g/2 scale is folded into the epilogue scale.
     * fc1: PE matmuls fp8 lhsT x bf16 rhs accumulated in PSUM fp32,
       ACT Gelu(psum * g1/2 + b1) -> bf16
     * fc2: PE matmuls, DVE epilogue psum * g2/2 + b2 -> fp32, DMA out.
 - Host: gather per-core y^T shards, transpose back.
"""
import sys

for _p in ("/root/.axon_site", "/root/.axon_site/_ro/trn_rl_repo",
           "/root/.axon_site/_ro/pypackages", "/opt/trn_rl_repo"):
    if _p not in sys.path:
        sys.path.append(_p)

import numpy as np

from concourse import bacc
import concourse.mybir as mybir
from concourse import bass_isa
from concourse.tile import TileContext
from concourse.bass_utils import run_bass_kernel_spmd

FP32 = mybir.dt.float32
BF16 = mybir.dt.bfloat16
FP8 = mybir.dt.float8e4
Act = mybir.ActivationFunctionType
Alu = mybir.AluOpType

N_CORES = 8
B, S, D, H = 64, 197, 768, 3072
TOK = B * S                 # 12608
TOK_PER = TOK // N_CORES    # 1576
NT = 4                      # token tiles per core
TN = TOK_PER // NT          # 394
KD = D // 128               # 6
KH = H // 128               # 24
EPS = 1e-5


def build(trace_scopes=False):
    nc = bacc.Bacc("TRN2", target_bir_lowering=False, debug=False)
    xt = nc.declare_dram_parameter("xt", [D, TOK_PER], FP32, isOutput=False)
    wt1 = nc.declare_dram_parameter("wt1", [D, H], FP32, isOutput=False)
    wt2 = nc.declare_dram_parameter("wt2", [H, D], FP32, isOutput=False)
    b1r = nc.declare_dram_parameter("b1r", [128, KH], FP32, isOutput=False)
    b2r = nc.declare_dram_parameter("b2r", [128, KD], FP32, isOutput=False)
    yt = nc.declare_dram_parameter("yt", [D, TOK_PER], FP32, isOutput=True)

    with TileContext(nc) as tc:
        with (
            tc.tile_pool(name="singles", bufs=1) as singles,
            tc.tile_pool(name="wres", bufs=KD) as wres,     # w1 fp32 resident
            tc.tile_pool(name="wf2", bufs=3) as wf2p,       # w2 fp32 stream
            tc.tile_pool(name="sgn", bufs=4) as sgnp,       # sign a/b transient
            tc.tile_pool(name="scr", bufs=2) as scrp,       # abs scratch
            tc.tile_pool(name="t1", bufs=KD) as t1p,        # ternary w1 fp8
            tc.tile_pool(name="t2", bufs=KH) as t2p,        # ternary w2 fp8
            tc.tile_pool(name="xb", bufs=KD) as xbp,        # x bf16 resident
            tc.tile_pool(name="hb", bufs=2 * KH + 4) as hbp,
            tc.tile_pool(name="ysb", bufs=4) as ysbp,
            tc.tile_pool(name="ps", bufs=5, space="PSUM") as psp,
            tc.tile_pool(name="ps2", bufs=3, space="PSUM") as ps2p,
        ):
            b1sb = singles.tile([128, KH], FP32, tag="b1sb")
            nc.sync.dma_start(out=b1sb, in_=b1r[:, :])
            b2sb = singles.tile([128, KD], FP32, tag="b2sb")
            nc.sync.dma_start(out=b2sb, in_=b2r[:, :])

            def gamma_chain(acc_cols, n_cols, total_elems, tag):
                # acc_cols [128, n_cols] of per-tile row sums -> g/2 bcast
                scr1 = singles.tile([128, n_cols], FP32, tag=tag + "_s1")
                rsum = singles.tile([128, 1], FP32, tag=tag + "_rs")
                nc.scalar.activation(scr1, acc_cols, Act.Abs,
                                     accum_out=rsum[:, 0:1])
                allr = singles.tile([128, 1], FP32, tag=tag + "_ar")
                nc.gpsimd.partition_all_reduce(allr, rsum, channels=128,
                                               reduce_op=bass_isa.ReduceOp.add)
                gh = singles.tile([128, 1], FP32, tag=tag + "_gh")
                nc.vector.tensor_scalar(
                    out=gh, in0=allr, scalar1=0.5 / total_elems,
                    scalar2=0.5 * EPS, op0=Alu.mult, op1=Alu.add)
                gn = singles.tile([128, 1], FP32, tag=tag + "_gn")
                nc.vector.tensor_scalar_mul(gn, gh, -1.0)
                return gh, gn

            # ---- w1 gamma pass (tiles resident) ----
            w1t = []
            acc1 = singles.tile([128, KD], FP32, tag="acc1")
            for kd in range(KD):
                wf = wres.tile([128, H], FP32, tag="w1")
                nc.sync.dma_start(out=wf, in_=wt1[kd * 128:(kd + 1) * 128, :])
                w1t.append(wf)
                scr = scrp.tile([128, H], BF16, tag="scr")
                nc.scalar.activation(scr, wf, Act.Abs,
                                     accum_out=acc1[:, kd:kd + 1])
            g1h, g1n = gamma_chain(acc1, KD, D * H, "g1")

            # ---- x load + bf16 cast via gpsimd DMA (parallel queue) ----
            xb = []
            for kd in range(KD):
                xbt = xbp.tile([128, TOK_PER], BF16, tag="xb")
                nc.gpsimd.dma_start(out=xbt, in_=xt[kd * 128:(kd + 1) * 128, :])
                xb.append(xbt)

            # ---- w1 quant: T = Sign(w+g/2)+Sign(w-g/2) in {-2,0,2} fp8 ----
            t1 = []
            for kd in range(KD):
                a = sgnp.tile([128, H], FP8, tag="sgn")
                b = sgnp.tile([128, H], FP8, tag="sgn")
                nc.scalar.activation(a, w1t[kd], Act.Sign, bias=g1h[:, 0:1])
                nc.scalar.activation(b, w1t[kd], Act.Sign, bias=g1n[:, 0:1])
                t = t1p.tile([128, H], FP8, tag="t1")
                nc.vector.tensor_add(out=t, in0=a, in1=b)
                t1.append(t)

            hbt = {}

            def fc1(t):
                tok = slice(t * TN, (t + 1) * TN)
                tiles = []
                for hc in range(KH):
                    ps = psp.tile([128, TN], FP32, tag="hps")
                    for kd in range(KD):
                        nc.tensor.matmul(
                            ps, t1[kd][:, hc * 128:(hc + 1) * 128],
                            xb[kd][:, tok],
                            start=(kd == 0), stop=(kd == KD - 1))
                    hbv = hbp.tile([128, TN], BF16, tag="hb")
                    nc.scalar.activation(hbv, ps, Act.Gelu,
                                         bias=b1sb[:, hc:hc + 1],
                                         scale=g1h[:, 0:1])
                    tiles.append(hbv)
                hbt[t] = tiles

            def fc2(t, t2, g2h):
                tok = slice(t * TN, (t + 1) * TN)
                for dc in range(KD):
                    ps2 = ps2p.tile([128, TN], FP32, tag="yps")
                    for kh in range(KH):
                        nc.tensor.matmul(
                            ps2, t2[kh][:, dc * 128:(dc + 1) * 128],
                            hbt[t][kh],
                            start=(kh == 0), stop=(kh == KH - 1))
                    ysb = ysbp.tile([128, TN], FP32, tag="ysb")
                    nc.vector.tensor_scalar(
                        out=ysb, in0=ps2, scalar1=g2h[:, 0:1],
                        scalar2=b2sb[:, dc:dc + 1],
                        op0=Alu.mult, op1=Alu.add)
                    nc.sync.dma_start(out=yt[dc * 128:(dc + 1) * 128, tok],
                                      in_=ysb)
                del hbt[t]

            # fc1(t0) first so its matmuls chase the w1 quant immediately
            fc1(0)

            # ---- w2 gamma pass (streamed) ----
            acc2 = singles.tile([128, KH], FP32, tag="acc2")
            for kh in range(KH):
                wf = wf2p.tile([128, D], FP32, tag="w2")
                nc.sync.dma_start(out=wf, in_=wt2[kh * 128:(kh + 1) * 128, :])
                scr = scrp.tile([128, D], BF16, tag="scr")
                nc.scalar.activation(scr, wf, Act.Abs,
                                     accum_out=acc2[:, kh:kh + 1])
            g2h, g2n = gamma_chain(acc2, KH, D * H, "g2")

            # ---- w2 quant (re-read) ----
            t2 = []
            for kh in range(KH):
                wf = wf2p.tile([128, D], FP32, tag="w2")
                nc.sync.dma_start(out=wf, in_=wt2[kh * 128:(kh + 1) * 128, :])
                a = sgnp.tile([128, D], FP8, tag="sgn")
                b = sgnp.tile([128, D], FP8, tag="sgn")
                nc.scalar.activation(a, wf, Act.Sign, bias=g2h[:, 0:1])
                nc.scalar.activation(b, wf, Act.Sign, bias=g2n[:, 0:1])
                t = t2p.tile([128, D], FP8, tag="t2")
                nc.vector.tensor_add(out=t, in0=a, in1=b)
                t2.append(t)

            # fc2 lags fc1 by one token tile so fc2(0) lands after w2 quant
            fc1(1)
            fc2(0, t2, g2h)
            fc1(2)
            fc2(1, t2, g2h)
            fc1(3)
            fc2(2, t2, g2h)
            fc2(3, t2, g2h)

    nc.compile()
    return nc


_NC = None


def _get_nc():
    global _NC
    if _NC is None:
        _NC = build()
    return _NC


def kernel(x, w1, b1, w2, b2, _trace=False, _trace_kwargs=None):
    nc = _get_nc()
    x2 = np.ascontiguousarray(x.reshape(TOK, D).T)      # [768, 12608]
    wt1 = np.ascontiguousarray(w1.T)                    # [768, 3072]
    wt2 = np.ascontiguousarray(w2.T)                    # [3072, 768]
    b1r = np.ascontiguousarray(b1.reshape(KH, 128).T)   # [128, 24]
    b2r = np.ascontiguousarray(b2.reshape(KD, 128).T)   # [128, 6]
    in_maps = []
    for c in range(N_CORES):
        in_maps.append({
            "xt": np.ascontiguousarray(x2[:, c * TOK_PER:(c + 1) * TOK_PER]),
            "wt1": wt1, "wt2": wt2, "b1r": b1r, "b2r": b2r,
        })
    out = run_bass_kernel_spmd(nc, in_maps, list(range(N_CORES)),
                               trace=_trace, **(_trace_kwargs or {}))
    res = out.results
    yt = np.concatenate([res[c]["yt"] for c in range(N_CORES)], axis=1)
    y = np.ascontiguousarray(yt.T).reshape(B, S, D).astype(np.float32)
    if _trace:
        return y, out
    return y


# revision 4
# speedup vs baseline: 1.1015x; 1.1015x over previous
"""Trainium2 Bass kernel for nn_Mlp_13099650253522 (BitNet-ternary dense MLP).

  h = gelu(x @ ter_quant(w1).T + b1);  y = h @ ter_quant(w2).T + b2
  ter_quant(w) = clip(round(w / g), -1, 1) * g,  g = mean(|w|) + 1e-5

Strategy (8 NeuronCores, data-parallel over the 64*197=12608 tokens):
 - Host: transpose x/w1/w2 (layout only), shard tokens 1576/core.
 - Device (per core, identical program):
     * gamma: ACT Abs+accum row sums -> GPSIMD partition_all_reduce
     * ternary quant, exact in fp8:
         w1 (head-critical): 4 tiles via ACT Sign(w+g/2)+Sign(w-g/2)
         in {-2,0,2} + 2 tiles via DVE (w>=g/2)*2 - (w<=-g/2)*2; scale g1/2
         w2 (overlaps fc1): DVE (w>=g/2) - (w<=-g/2) in {-1,0,1}; scale g2
     * fc1: PE matmul fp8 lhsT x bf16 rhs -> PSUM fp32;
       ACT Gelu(psum * g1/2 + b1) -> bf16
     * fc2: PE matmuls; DVE epilogue psum * g2 + b2 -> fp32; DMA out
 - DMA streams are serialized w1 -> x -> w2(pass1) -> w2(pass2) with
   explicit deps so the head-critical w1 read gets full HBM bandwidth.
 - Host: gather per-core y^T shards, transpose back.
"""
import sys

for _p in ("/root/.axon_site", "/root/.axon_site/_ro/trn_rl_repo",
           "/root/.axon_site/_ro/pypackages", "/opt/trn_rl_repo"):
    if _p not in sys.path:
        sys.path.append(_p)

import numpy as np

from concourse import bacc
import concourse.mybir as mybir
from concourse import bass_isa
from concourse.tile import TileContext
from concourse.tile_rust import add_dep_helper
from concourse.bass_utils import run_bass_kernel_spmd

FP32 = mybir.dt.float32
BF16 = mybir.dt.bfloat16
FP8 = mybir.dt.float8e4
Act = mybir.ActivationFunctionType
Alu = mybir.AluOpType

N_CORES = 8
B, S, D, H = 64, 197, 768, 3072
TOK = B * S                 # 12608
TOK_PER = TOK // N_CORES    # 1576
NT = 4                      # token tiles per core
TN = TOK_PER // NT          # 394
KD = D // 128               # 6
KH = H // 128               # 24
EPS = 1e-5
W1_ACT_TILES = 4            # w1 tiles quantized on ACT; rest on DVE


def build():
    nc = bacc.Bacc("TRN2", target_bir_lowering=False, debug=False)
    xt = nc.declare_dram_parameter("xt", [D, TOK_PER], FP32, isOutput=False)
    wt1 = nc.declare_dram_parameter("wt1", [D, H], FP32, isOutput=False)
    wt2 = nc.declare_dram_parameter("wt2", [H, D], FP32, isOutput=False)
    b1r = nc.declare_dram_parameter("b1r", [128, KH], FP32, isOutput=False)
    b2r = nc.declare_dram_parameter("b2r", [128, KD], FP32, isOutput=False)
    yt = nc.declare_dram_parameter("yt", [D, TOK_PER], FP32, isOutput=True)

    with TileContext(nc) as tc:
        with (
            tc.tile_pool(name="singles", bufs=1) as singles,
            tc.tile_pool(name="wres", bufs=KD) as wres,     # w1 fp32 resident
            tc.tile_pool(name="wf2", bufs=3) as wf2p,       # w2 fp32 stream
            tc.tile_pool(name="sgn", bufs=4) as sgnp,       # quant transients
            tc.tile_pool(name="scr", bufs=2) as scrp,       # abs scratch
            tc.tile_pool(name="t1", bufs=KD) as t1p,        # ternary w1 fp8
            tc.tile_pool(name="t2", bufs=KH) as t2p,        # ternary w2 fp8
            tc.tile_pool(name="xb", bufs=KD) as xbp,        # x bf16 resident
            tc.tile_pool(name="hb", bufs=2 * KH + 2) as hbp,
            tc.tile_pool(name="ysb", bufs=4) as ysbp,
            tc.tile_pool(name="ps", bufs=5, space="PSUM") as psp,
            tc.tile_pool(name="ps2", bufs=3, space="PSUM") as ps2p,
        ):
            b1sb = singles.tile([128, KH], FP32, tag="b1sb")
            nc.sync.dma_start(out=b1sb, in_=b1r[:, :])
            b2sb = singles.tile([128, KD], FP32, tag="b2sb")
            nc.sync.dma_start(out=b2sb, in_=b2r[:, :])

            def gamma_chain(acc_cols, n_cols, total_elems, tag):
                """per-tile row sums [128,n] -> (g/2, -g/2, g) bcast [128,1]"""
                scr1 = singles.tile([128, n_cols], FP32, tag=tag + "_s1")
                rsum = singles.tile([128, 1], FP32, tag=tag + "_rs")
                nc.scalar.activation(scr1, acc_cols, Act.Abs,
                                     accum_out=rsum[:, 0:1])
                allr = singles.tile([128, 1], FP32, tag=tag + "_ar")
                nc.gpsimd.partition_all_reduce(allr, rsum, channels=128,
                                               reduce_op=bass_isa.ReduceOp.add)
                gf = singles.tile([128, 1], FP32, tag=tag + "_gf")
                nc.vector.tensor_scalar(
                    out=gf, in0=allr, scalar1=1.0 / total_elems,
                    scalar2=EPS, op0=Alu.mult, op1=Alu.add)
                gh = singles.tile([128, 1], FP32, tag=tag + "_gh")
                nc.vector.tensor_scalar_mul(gh, gf, 0.5)
                gn = singles.tile([128, 1], FP32, tag=tag + "_gn")
                nc.vector.tensor_scalar_mul(gn, gf, -0.5)
                return gh, gn, gf

            # ---- w1 gamma pass (tiles resident; full HBM BW) ----
            w1t = []
            w1_dmas = []
            acc1 = singles.tile([128, KD], FP32, tag="acc1")
            for kd in range(KD):
                wf = wres.tile([128, H], FP32, tag="w1")
                w1_dmas.append(
                    nc.sync.dma_start(out=wf,
                                      in_=wt1[kd * 128:(kd + 1) * 128, :]))
                w1t.append(wf)
                scr = scrp.tile([128, H], BF16, tag="scr")
                nc.scalar.activation(scr, wf, Act.Abs,
                                     accum_out=acc1[:, kd:kd + 1])
            g1h, g1n, _ = gamma_chain(acc1, KD, D * H, "g1")

            # ---- x load + bf16 cast (gated behind the w1 read) ----
            xb = []
            xb_dmas = []
            for kd in range(KD):
                xbt = xbp.tile([128, TOK_PER], BF16, tag="xb")
                dma = nc.gpsimd.dma_start(out=xbt,
                                          in_=xt[kd * 128:(kd + 1) * 128, :])
                add_dep_helper(dma.ins, w1_dmas[-1].ins, reason="dma order: x after w1")
                xb_dmas.append(dma)
                xb.append(xbt)

            # ---- w1 quant -> T1 in {-2,0,2} fp8; scale g1/2 ----
            t1 = []
            for kd in range(KD):
                t = t1p.tile([128, H], FP8, tag="t1")
                if kd < W1_ACT_TILES:
                    a = sgnp.tile([128, H], FP8, tag="sgn")
                    b = sgnp.tile([128, H], FP8, tag="sgn")
                    nc.scalar.activation(a, w1t[kd], Act.Sign, bias=g1h[:, 0:1])
                    nc.scalar.activation(b, w1t[kd], Act.Sign, bias=g1n[:, 0:1])
                    nc.vector.tensor_add(out=t, in0=a, in1=b)
                else:
                    a = sgnp.tile([128, H], FP8, tag="sgn")
                    b = sgnp.tile([128, H], FP8, tag="sgn")
                    nc.vector.tensor_scalar(
                        out=a, in0=w1t[kd], scalar1=g1h[:, 0:1], scalar2=2.0,
                        op0=Alu.is_ge, op1=Alu.mult)
                    nc.vector.tensor_scalar(
                        out=b, in0=w1t[kd], scalar1=g1n[:, 0:1], scalar2=2.0,
                        op0=Alu.is_le, op1=Alu.mult)
                    nc.vector.tensor_sub(out=t, in0=a, in1=b)
                t1.append(t)

            hbt = {}

            def fc1(t):
                tok = slice(t * TN, (t + 1) * TN)
                tiles = []
                for hc in range(KH):
                    ps = psp.tile([128, TN], FP32, tag="hps")
                    order = [(hc + j) % KD for j in range(KD)]
                    for j, kd in enumerate(order):
                        nc.tensor.matmul(
                            ps, t1[kd][:, hc * 128:(hc + 1) * 128],
                            xb[kd][:, tok],
                            start=(j == 0), stop=(j == KD - 1))
                    hbv = hbp.tile([128, TN], BF16, tag="hb")
                    nc.scalar.activation(hbv, ps, Act.Gelu,
                                         bias=b1sb[:, hc:hc + 1],
                                         scale=g1h[:, 0:1])
                    tiles.append(hbv)
                hbt[t] = tiles

            def fc2(t, t2, g2f):
                tok = slice(t * TN, (t + 1) * TN)
                for dc in range(KD):
                    ps2 = ps2p.tile([128, TN], FP32, tag="yps")
                    order = [(dc + j) % KH for j in range(KH)]
                    for j, kh in enumerate(order):
                        nc.tensor.matmul(
                            ps2, t2[kh][:, dc * 128:(dc + 1) * 128],
                            hbt[t][kh],
                            start=(j == 0), stop=(j == KH - 1))
                    ysb = ysbp.tile([128, TN], FP32, tag="ysb")
                    nc.vector.tensor_scalar(
                        out=ysb, in0=ps2, scalar1=g2f[:, 0:1],
                        scalar2=b2sb[:, dc:dc + 1],
                        op0=Alu.mult, op1=Alu.add)
                    nc.sync.dma_start(out=yt[dc * 128:(dc + 1) * 128, tok],
                                      in_=ysb)
                del hbt[t]

            # fc1(t0) chases the w1 quant immediately
            fc1(0)

            # ---- w2 gamma pass (streamed; gated behind x load) ----
            acc2 = singles.tile([128, KH], FP32, tag="acc2")
            w2p1_dmas = []
            for kh in range(KH):
                wf = wf2p.tile([128, D], FP32, tag="w2")
                dma = nc.sync.dma_start(out=wf,
                                        in_=wt2[kh * 128:(kh + 1) * 128, :])
                add_dep_helper(dma.ins, xb_dmas[-1].ins, reason="dma order: w2p1 after x")
                w2p1_dmas.append(dma)
                scr = scrp.tile([128, D], BF16, tag="scr")
                nc.scalar.activation(scr, wf, Act.Abs,
                                     accum_out=acc2[:, kh:kh + 1])
            g2h, g2n, g2f = gamma_chain(acc2, KH, D * H, "g2")

            # ---- w2 quant on DVE (re-read, gated) -> {-1,0,1}; scale g2 ----
            t2 = []
            for kh in range(KH):
                wf = wf2p.tile([128, D], FP32, tag="w2")
                dma = nc.sync.dma_start(out=wf,
                                        in_=wt2[kh * 128:(kh + 1) * 128, :])
                add_dep_helper(dma.ins, w2p1_dmas[-1].ins,
                               reason="dma order: w2p2 after w2p1")
                a = sgnp.tile([128, D], FP8, tag="sgn")
                b = sgnp.tile([128, D], FP8, tag="sgn")
                nc.vector.tensor_scalar(out=a, in0=wf, scalar1=g2h[:, 0:1],
                                        scalar2=None, op0=Alu.is_ge)
                nc.vector.tensor_scalar(out=b, in0=wf, scalar1=g2n[:, 0:1],
                                        scalar2=None, op0=Alu.is_le)
                t = t2p.tile([128, D], FP8, tag="t2")
                nc.vector.tensor_sub(out=t, in0=a, in1=b)
                t2.append(t)

            # fc2 lags fc1 by one token tile
            fc1(1)
            fc2(0, t2, g2f)
            fc1(2)
            fc2(1, t2, g2f)
            fc1(3)
            fc2(2, t2, g2f)
            fc2(3, t2, g2f)

    nc.compile()
    return nc


_NC = None


def _get_nc():
    global _NC
    if _NC is None:
        _NC = build()
    return _NC


def kernel(x, w1, b1, w2, b2, _trace=False, _trace_kwargs=None):
    nc = _get_nc()
    x2 = np.ascontiguousarray(x.reshape(TOK, D).T)      # [768, 12608]
    wt1 = np.ascontiguousarray(w1.T)                    # [768, 3072]
    wt2 = np.ascontiguousarray(w2.T)                    # [3072, 768]
    b1r = np.ascontiguousarray(b1.reshape(KH, 128).T)   # [128, 24]
    b2r = np.ascontiguousarray(b2.reshape(KD, 128).T)   # [128, 6]
    in_maps = []
    for c in range(N_CORES):
        in_maps.append({
            "xt": np.ascontiguousarray(x2[:, c * TOK_PER:(c + 1) * TOK_PER]),
            "wt1": wt1, "wt2": wt2, "b1r": b1r, "b2r": b2r,
        })
    out = run_bass_kernel_spmd(nc, in_maps, list(range(N_CORES)),
                               trace=_trace, **(_trace_kwargs or {}))
    res = out.results
    yt = np.concatenate([res[c]["yt"] for c in range(N_CORES)], axis=1)
    y = np.ascontiguousarray(yt.T).reshape(B, S, D).astype(np.float32)
    if _trace:
        return y, out
    return y


# revision 7
# speedup vs baseline: 1.1633x; 1.0560x over previous
"""Trainium2 Bass kernel for nn_Mlp_13099650253522 (BitNet-ternary dense MLP).

  h = gelu(x @ ter_quant(w1).T + b1);  y = h @ ter_quant(w2).T + b2
  ter_quant(w) = clip(round(w / g), -1, 1) * g,  g = mean(|w|) + 1e-5

Strategy (8 NeuronCores, data-parallel over the 64*197=12608 tokens):
 - Host: transpose x/w1/w2 (layout only), shard tokens 1576/core.
 - Device (per core, identical program):
     * gamma: DVE tensor_reduce(|w|) row sums -> GPSIMD partition_all_reduce
       (library pre-warmed with a dummy op at kernel start)
     * ternary quant, exact in fp8 {-2,0,+2}; g/2 folded into epilogues:
       ACT path: Sign(w+g/2)+Sign(w-g/2); DVE path: (w>=g/2)*2-(w<=-g/2)*2,
       split across both engines to shorten the critical path
     * fc1: PE matmul fp8 lhsT x bf16 rhs -> PSUM fp32;
       ACT Gelu(psum * g1/2 + b1) -> bf16
     * fc2: PE matmuls; DVE epilogue psum * g2/2 + b2 -> fp32; DMA out
 - DMA streams serialized w1 -> x -> w2(pass1) -> w2(pass2) via explicit
   deps; w2 moves in 6 big batches per pass for full HBM bandwidth.
 - Host: gather per-core y^T shards, transpose back.
"""
import sys

for _p in ("/root/.axon_site", "/root/.axon_site/_ro/trn_rl_repo",
           "/root/.axon_site/_ro/pypackages", "/opt/trn_rl_repo"):
    if _p not in sys.path:
        sys.path.append(_p)

import numpy as np

from concourse import bacc
import concourse.mybir as mybir
from concourse import bass_isa
from concourse.tile import TileContext
from concourse.tile_rust import add_dep_helper
from concourse.bass_utils import run_bass_kernel_spmd

FP32 = mybir.dt.float32
BF16 = mybir.dt.bfloat16
FP8 = mybir.dt.float8e4
Act = mybir.ActivationFunctionType
Alu = mybir.AluOpType
AxX = mybir.AxisListType.X

N_CORES = 8
B, S, D, H = 64, 197, 768, 3072
TOK = B * S                 # 12608
TOK_PER = TOK // N_CORES    # 1576
NT = 4                      # token tiles per core
TN = TOK_PER // NT          # 394
KD = D // 128               # 6
KH = H // 128               # 24
EPS = 1e-5

W1C = 12                    # w1 load chunks: [128, 1536], 2 per kd
W1_ACT_CHUNKS = 8           # of those, how many quantized on ACT
W2B = 6                     # w2 batches: [128, 3072], 4 kh per batch
W2_ACT_BATCHES = 2          # of those, how many quantized on ACT


def build():
    nc = bacc.Bacc("TRN2", target_bir_lowering=False, debug=False)
    xt = nc.declare_dram_parameter("xt", [D, TOK_PER], FP32, isOutput=False)
    wt1 = nc.declare_dram_parameter("wt1", [D, H], FP32, isOutput=False)
    wt2 = nc.declare_dram_parameter("wt2", [H, D], FP32, isOutput=False)
    b1r = nc.declare_dram_parameter("b1r", [128, KH], FP32, isOutput=False)
    b2r = nc.declare_dram_parameter("b2r", [128, KD], FP32, isOutput=False)
    yt = nc.declare_dram_parameter("yt", [D, TOK_PER], FP32, isOutput=True)

    with TileContext(nc) as tc:
        with (
            tc.tile_pool(name="singles", bufs=1) as singles,
            tc.tile_pool(name="wres", bufs=W1C) as wres,    # w1 fp32 resident
            tc.tile_pool(name="wf2", bufs=2) as wf2p,       # w2 fp32 stream
            tc.tile_pool(name="sgn", bufs=3) as sgnp,       # quant transients
            tc.tile_pool(name="t1", bufs=W1C) as t1p,       # ternary w1 fp8
            tc.tile_pool(name="t2", bufs=W2B) as t2p,       # ternary w2 fp8
            tc.tile_pool(name="xb", bufs=KD) as xbp,        # x bf16 resident
            tc.tile_pool(name="hb", bufs=52) as hbp,
            tc.tile_pool(name="ysb", bufs=4) as ysbp,
            tc.tile_pool(name="ps", bufs=5, space="PSUM") as psp,
            tc.tile_pool(name="ps2", bufs=3, space="PSUM") as ps2p,
        ):
            # warm the gpsimd custom-op library while w1 streams in
            dmy = singles.tile([128, 1], FP32, tag="dmy")
            nc.gpsimd.memset(dmy, 0.0)
            dmy2 = singles.tile([128, 1], FP32, tag="dmy2")
            nc.gpsimd.partition_all_reduce(dmy2, dmy, channels=128,
                                           reduce_op=bass_isa.ReduceOp.add)

            def gamma_chain(acc_cols, n_cols, total_elems, tag):
                """per-tile |row| sums [128,n] -> (g/2, -g/2) bcast [128,1]"""
                rsum = singles.tile([128, 1], FP32, tag=tag + "_rs")
                nc.vector.tensor_reduce(out=rsum[:, 0:1], in_=acc_cols,
                                        axis=AxX, op=Alu.add)
                allr = singles.tile([128, 1], FP32, tag=tag + "_ar")
                nc.gpsimd.partition_all_reduce(allr, rsum, channels=128,
                                               reduce_op=bass_isa.ReduceOp.add)
                gf = singles.tile([128, 1], FP32, tag=tag + "_gf")
                nc.vector.tensor_scalar(
                    out=gf, in0=allr, scalar1=1.0 / total_elems,
                    scalar2=EPS, op0=Alu.mult, op1=Alu.add)
                gh = singles.tile([128, 1], FP32, tag=tag + "_gh")
                nc.vector.tensor_scalar_mul(gh, gf, 0.5)
                gn = singles.tile([128, 1], FP32, tag=tag + "_gn")
                nc.vector.tensor_scalar_mul(gn, gf, -0.5)
                return gh, gn

            # ---- w1 gamma pass: 12 chunks [128,1536], resident ----
            HC2 = H // 2
            w1t = []
            w1_dmas = []
            acc1 = singles.tile([128, W1C], FP32, tag="acc1")
            for c in range(W1C):
                kd, half = c // 2, c % 2
                wf = wres.tile([128, HC2], FP32, tag="w1")
                w1_dmas.append(nc.sync.dma_start(
                    out=wf, in_=wt1[kd * 128:(kd + 1) * 128,
                                    half * HC2:(half + 1) * HC2]))
                w1t.append(wf)
                nc.vector.tensor_reduce(out=acc1[:, c:c + 1], in_=wf,
                                        axis=AxX, op=Alu.add,
                                        apply_absolute_value=True)
            g1h, g1n = gamma_chain(acc1, W1C, D * H, "g1")

            b1sb = singles.tile([128, KH], FP32, tag="b1sb")
            nc.sync.dma_start(out=b1sb, in_=b1r[:, :])
            b2sb = singles.tile([128, KD], FP32, tag="b2sb")
            nc.sync.dma_start(out=b2sb, in_=b2r[:, :])

            # ---- x load + bf16 cast (gpsimd queue, gated after w1) ----
            xb = []
            xb_dmas = []
            for kd in range(KD):
                xbt = xbp.tile([128, TOK_PER], BF16, tag="xb")
                dma = nc.gpsimd.dma_start(out=xbt,
                                          in_=xt[kd * 128:(kd + 1) * 128, :])
                add_dep_helper(dma.ins, w1_dmas[-1].ins,
                               reason="dma order: x after w1")
                xb_dmas.append(dma)
                xb.append(xbt)

            def quant_act(wf, t, gh, gn):
                a = sgnp.tile(list(wf.shape), FP8, tag="sgn")
                b = sgnp.tile(list(wf.shape), FP8, tag="sgn")
                nc.scalar.activation(a, wf, Act.Sign, bias=gh[:, 0:1])
                nc.scalar.activation(b, wf, Act.Sign, bias=gn[:, 0:1])
                nc.vector.tensor_add(out=t, in0=a, in1=b)

            def quant_dve(wf, t, gh, gn):
                a = sgnp.tile(list(wf.shape), FP8, tag="sgn")
                b = sgnp.tile(list(wf.shape), FP8, tag="sgn")
                nc.vector.tensor_scalar(out=a, in0=wf, scalar1=gh[:, 0:1],
                                        scalar2=2.0, op0=Alu.is_ge,
                                        op1=Alu.mult)
                nc.vector.tensor_scalar(out=b, in0=wf, scalar1=gn[:, 0:1],
                                        scalar2=2.0, op0=Alu.is_le,
                                        op1=Alu.mult)
                nc.vector.tensor_sub(out=t, in0=a, in1=b)

            # ---- w1 quant -> T1 {-2,0,2} fp8, scale g1/2 ----
            t1 = []
            for c in range(W1C):
                t = t1p.tile([128, HC2], FP8, tag="t1")
                if c < W1_ACT_CHUNKS:
                    quant_act(w1t[c], t, g1h, g1n)
                else:
                    quant_dve(w1t[c], t, g1h, g1n)
                t1.append(t)

            def t1_slice(hc, kd):
                # lhsT [128,128] for fc1 group hc, contraction chunk kd
                c = kd * 2 + (hc * 128) // HC2
                off = (hc * 128) % HC2
                return t1[c][:, off:off + 128]

            hbt = {}

            def fc1(t, hcs=range(KH)):
                tok = slice(t * TN, (t + 1) * TN)
                for hc in hcs:
                    ps = psp.tile([128, TN], FP32, tag="hps")
                    order = [(hc + j) % KD for j in range(KD)]
                    for j, kd in enumerate(order):
                        nc.tensor.matmul(ps, t1_slice(hc, kd), xb[kd][:, tok],
                                         start=(j == 0), stop=(j == KD - 1))
                    hbv = hbp.tile([128, TN], BF16, tag="hb")
                    nc.scalar.activation(hbv, ps, Act.Gelu,
                                         bias=b1sb[:, hc:hc + 1],
                                         scale=g1h[:, 0:1])
                    hbt.setdefault(t, []).append(hbv)

            def fc2(t, t2s, g2h):
                tok = slice(t * TN, (t + 1) * TN)
                for dc in range(KD):
                    ps2 = ps2p.tile([128, TN], FP32, tag="yps")
                    order = [(dc + j) % KH for j in range(KH)]
                    for j, kh in enumerate(order):
                        lhsT = t2s[kh // 4][:, kh % 4,
                                            dc * 128:(dc + 1) * 128]
                        nc.tensor.matmul(ps2, lhsT, hbt[t][kh],
                                         start=(j == 0), stop=(j == KH - 1))
                    ysb = ysbp.tile([128, TN], FP32, tag="ysb")
                    nc.vector.tensor_scalar(
                        out=ysb, in0=ps2, scalar1=g2h[:, 0:1],
                        scalar2=b2sb[:, dc:dc + 1],
                        op0=Alu.mult, op1=Alu.add)
                    nc.sync.dma_start(out=yt[dc * 128:(dc + 1) * 128, tok],
                                      in_=ysb)
                del hbt[t]

            # ---- fc1(t0) chases the w1 quant ----
            fc1(0)

            # ---- w2 pass 1: 6 batches [128,3072] (4 kh each), gated ----
            acc2 = singles.tile([128, W2B], FP32, tag="acc2")
            w2p1_dmas = []
            for bt in range(W2B):
                wf = wf2p.tile([128, 4, D], FP32, tag="w2")
                src = wt2[bt * 512:(bt + 1) * 512, :]
                dma = nc.sync.dma_start(
                    out=wf, in_=src.rearrange("(c p) f -> p c f", p=128))
                add_dep_helper(dma.ins, xb_dmas[-1].ins,
                               reason="dma order: w2p1 after x")
                w2p1_dmas.append(dma)
                nc.vector.tensor_reduce(out=acc2[:, bt:bt + 1], in_=wf,
                                        axis=mybir.AxisListType.XY, op=Alu.add,
                                        apply_absolute_value=True)
            g2h, g2n = gamma_chain(acc2, W2B, D * H, "g2")

            # ---- w2 pass 2 (re-read) + quant -> {-2,0,2}, scale g2/2 ----
            t2 = [None] * W2B

            def w2_quant_batch(bt):
                wf = wf2p.tile([128, 4, D], FP32, tag="w2")
                src = wt2[bt * 512:(bt + 1) * 512, :]
                dma = nc.sync.dma_start(
                    out=wf, in_=src.rearrange("(c p) f -> p c f", p=128))
                add_dep_helper(dma.ins, w2p1_dmas[-1].ins,
                               reason="dma order: w2p2 after w2p1")
                t = t2p.tile([128, 4, D], FP8, tag="t2")
                if bt < W2_ACT_BATCHES:
                    quant_act(wf, t, g2h, g2n)
                else:
                    quant_dve(wf, t, g2h, g2n)
                t2[bt] = t

            # interleave ACT-path w2 quant between fc1(1) GELU emission
            fc1(1, range(0, 8))
            w2_quant_batch(0)
            fc1(1, range(8, 16))
            w2_quant_batch(1)
            fc1(1, range(16, 24))
            for bt in range(2, W2B):
                w2_quant_batch(bt)

            fc2(0, t2, g2h)
            fc1(2)
            fc2(1, t2, g2h)
            fc1(3)
            fc2(2, t2, g2h)
            fc2(3, t2, g2h)

    nc.compile()
    return nc


_NC = None


def _get_nc():
    global _NC
    if _NC is None:
        _NC = build()
    return _NC


def kernel(x, w1, b1, w2, b2, _trace=False, _trace_kwargs=None):
    nc = _get_nc()
    x2 = np.ascontiguousarray(x.reshape(TOK, D).T)      # [768, 12608]
    wt1 = np.ascontiguousarray(w1.T)                    # [768, 3072]
    wt2 = np.ascontiguousarray(w2.T)                    # [3072, 768]
    b1r = np.ascontiguousarray(b1.reshape(KH, 128).T)   # [128, 24]
    b2r = np.ascontiguousarray(b2.reshape(KD, 128).T)   # [128, 6]
    in_maps = []
    for c in range(N_CORES):
        in_maps.append({
            "xt": np.ascontiguousarray(x2[:, c * TOK_PER:(c + 1) * TOK_PER]),
            "wt1": wt1, "wt2": wt2, "b1r": b1r, "b2r": b2r,
        })
    out = run_bass_kernel_spmd(nc, in_maps, list(range(N_CORES)),
                               trace=_trace, **(_trace_kwargs or {}))
    res = out.results
    yt = np.concatenate([res[c]["yt"] for c in range(N_CORES)], axis=1)
    y = np.ascontiguousarray(yt.T).reshape(B, S, D).astype(np.float32)
    if _trace:
        return y, out
    return y


# revision 8
# speedup vs baseline: 1.2083x; 1.0387x over previous
"""Trainium2 Bass kernel for nn_Mlp_13099650253522 (BitNet-ternary dense MLP).

  h = gelu(x @ ter_quant(w1).T + b1);  y = h @ ter_quant(w2).T + b2
  ter_quant(w) = clip(round(w / g), -1, 1) * g,  g = mean(|w|) + 1e-5

Strategy (8 NeuronCores, data-parallel over the 64*197=12608 tokens):
 - Host: transpose x/w1/w2 (layout only), shard tokens 1576/core.
 - Device (per core, identical program):
     * gamma: DVE tensor_reduce(|w|) row sums -> GPSIMD partition_all_reduce
       (library pre-warmed with a dummy op at kernel start)
     * ternary quant, exact in fp8 {-2,0,+2}; g/2 folded into epilogues:
       ACT path: Sign(w+g/2)+Sign(w-g/2); DVE path: (w>=g/2)*2-(w<=-g/2)*2,
       split across both engines to shorten the critical path
     * fc1: PE matmul fp8 lhsT x bf16 rhs -> PSUM fp32;
       ACT Gelu(psum * g1/2 + b1) -> bf16
     * fc2: PE matmuls; DVE epilogue psum * g2/2 + b2 -> fp32; DMA out
 - DMA streams serialized w1 -> x -> w2(pass1) -> w2(pass2) via explicit
   deps; w2 moves in 6 big batches per pass for full HBM bandwidth.
 - Host: gather per-core y^T shards, transpose back.
"""
import sys

for _p in ("/root/.axon_site", "/root/.axon_site/_ro/trn_rl_repo",
           "/root/.axon_site/_ro/pypackages", "/opt/trn_rl_repo"):
    if _p not in sys.path:
        sys.path.append(_p)

import numpy as np

from concourse import bacc
import concourse.mybir as mybir
from concourse import bass_isa
from concourse.tile import TileContext
from concourse.tile_rust import add_dep_helper
from concourse.bass_utils import run_bass_kernel_spmd

FP32 = mybir.dt.float32
BF16 = mybir.dt.bfloat16
FP8 = mybir.dt.float8e4
Act = mybir.ActivationFunctionType
Alu = mybir.AluOpType
AxX = mybir.AxisListType.X

N_CORES = 8
B, S, D, H = 64, 197, 768, 3072
TOK = B * S                 # 12608
TOK_PER = TOK // N_CORES    # 1576
NT = 4                      # token tiles per core
TN = TOK_PER // NT          # 394
KD = D // 128               # 6
KH = H // 128               # 24
EPS = 1e-5

W1C = 12                    # w1 load chunks: [128, 1536], 2 per kd
W1_DVE_CHUNKS = 4           # first chunks quantized on DVE (emitted first)
W2B = 6                     # w2 batches: [128, 3072], 4 kh per batch
W2_ACT_BATCHES = 2          # of those, how many quantized on ACT


def build():
    nc = bacc.Bacc("TRN2", target_bir_lowering=False, debug=False)
    xt = nc.declare_dram_parameter("xt", [D, TOK_PER], FP32, isOutput=False)
    wt1 = nc.declare_dram_parameter("wt1", [D, H], FP32, isOutput=False)
    wt2 = nc.declare_dram_parameter("wt2", [H, D], FP32, isOutput=False)
    b1r = nc.declare_dram_parameter("b1r", [128, KH], FP32, isOutput=False)
    b2r = nc.declare_dram_parameter("b2r", [128, KD], FP32, isOutput=False)
    yt = nc.declare_dram_parameter("yt", [D, TOK_PER], FP32, isOutput=True)

    with TileContext(nc) as tc:
        with (
            tc.tile_pool(name="singles", bufs=1) as singles,
            tc.tile_pool(name="wres", bufs=W1C) as wres,    # w1 fp32 resident
            tc.tile_pool(name="wf2", bufs=3) as wf2p,       # w2 fp32 stream
            tc.tile_pool(name="sgn", bufs=3) as sgnp,       # quant transients
            tc.tile_pool(name="t1", bufs=W1C) as t1p,       # ternary w1 fp8
            tc.tile_pool(name="xb", bufs=KD) as xbp,        # x bf16 resident
            tc.tile_pool(name="hb", bufs=52) as hbp,
            tc.tile_pool(name="ysb", bufs=4) as ysbp,
            tc.tile_pool(name="ps", bufs=5, space="PSUM") as psp,
            tc.tile_pool(name="ps2", bufs=3, space="PSUM") as ps2p,
        ):
            # warm the gpsimd custom-op library while w1 streams in
            dmy = singles.tile([128, 1], FP32, tag="dmy")
            nc.gpsimd.memset(dmy, 0.0)
            dmy2 = singles.tile([128, 1], FP32, tag="dmy2")
            nc.gpsimd.partition_all_reduce(dmy2, dmy, channels=128,
                                           reduce_op=bass_isa.ReduceOp.add)

            def gamma_chain(acc_cols, n_cols, total_elems, tag):
                """per-tile |row| sums [128,n] -> (g/2, -g/2) bcast [128,1]"""
                rsum = singles.tile([128, 1], FP32, tag=tag + "_rs")
                nc.vector.tensor_reduce(out=rsum[:, 0:1], in_=acc_cols,
                                        axis=AxX, op=Alu.add)
                allr = singles.tile([128, 1], FP32, tag=tag + "_ar")
                nc.gpsimd.partition_all_reduce(allr, rsum, channels=128,
                                               reduce_op=bass_isa.ReduceOp.add)
                gf = singles.tile([128, 1], FP32, tag=tag + "_gf")
                nc.vector.tensor_scalar(
                    out=gf, in0=allr, scalar1=1.0 / total_elems,
                    scalar2=EPS, op0=Alu.mult, op1=Alu.add)
                gh = singles.tile([128, 1], FP32, tag=tag + "_gh")
                nc.vector.tensor_scalar_mul(gh, gf, 0.5)
                gn = singles.tile([128, 1], FP32, tag=tag + "_gn")
                nc.vector.tensor_scalar_mul(gn, gf, -0.5)
                return gh, gn

            # ---- w1 gamma pass: 12 chunks [128,1536], resident ----
            HC2 = H // 2
            w1t = []
            w1_dmas = []
            acc1 = singles.tile([128, W1C], FP32, tag="acc1")
            for c in range(W1C):
                kd, half = c // 2, c % 2
                wf = wres.tile([128, HC2], FP32, tag="w1")
                w1_dmas.append(nc.sync.dma_start(
                    out=wf, in_=wt1[kd * 128:(kd + 1) * 128,
                                    half * HC2:(half + 1) * HC2]))
                w1t.append(wf)
                nc.vector.tensor_reduce(out=acc1[:, c:c + 1], in_=wf,
                                        axis=AxX, op=Alu.add,
                                        apply_absolute_value=True)
            g1h, g1n = gamma_chain(acc1, W1C, D * H, "g1")

            b1sb = singles.tile([128, KH], FP32, tag="b1sb")
            nc.sync.dma_start(out=b1sb, in_=b1r[:, :])
            b2sb = singles.tile([128, KD], FP32, tag="b2sb")
            nc.sync.dma_start(out=b2sb, in_=b2r[:, :])

            # ---- x load + bf16 cast (gpsimd queue, gated after w1) ----
            xb = []
            xb_dmas = []
            for kd in range(KD):
                xbt = xbp.tile([128, TOK_PER], BF16, tag="xb")
                dma = nc.gpsimd.dma_start(out=xbt,
                                          in_=xt[kd * 128:(kd + 1) * 128, :])
                add_dep_helper(dma.ins, w1_dmas[-1].ins,
                               reason="dma order: x after w1")
                xb_dmas.append(dma)
                xb.append(xbt)

            def quant_act(wf, t, gh, gn):
                a = sgnp.tile(list(wf.shape), FP8, tag="sgn")
                b = sgnp.tile(list(wf.shape), FP8, tag="sgn")
                nc.scalar.activation(a, wf, Act.Sign, bias=gh[:, 0:1])
                nc.scalar.activation(b, wf, Act.Sign, bias=gn[:, 0:1])
                nc.vector.tensor_add(out=t, in0=a, in1=b)

            def quant_dve(wf, t, gh, gn):
                a = sgnp.tile(list(wf.shape), FP8, tag="sgn")
                b = sgnp.tile(list(wf.shape), FP8, tag="sgn")
                nc.vector.tensor_scalar(out=a, in0=wf, scalar1=gh[:, 0:1],
                                        scalar2=2.0, op0=Alu.is_ge,
                                        op1=Alu.mult)
                nc.vector.tensor_scalar(out=b, in0=wf, scalar1=gn[:, 0:1],
                                        scalar2=2.0, op0=Alu.is_le,
                                        op1=Alu.mult)
                nc.vector.tensor_sub(out=t, in0=a, in1=b)

            # ---- w1 quant -> T1 {-2,0,2} fp8, scale g1/2 ----
            t1 = []
            for c in range(W1C):
                t = t1p.tile([128, HC2], FP8, tag="t1")
                if c < W1_DVE_CHUNKS:
                    quant_dve(w1t[c], t, g1h, g1n)
                else:
                    quant_act(w1t[c], t, g1h, g1n)
                t1.append(t)

            def t1_slice(hc, kd):
                # lhsT [128,128] for fc1 group hc, contraction chunk kd
                c = kd * 2 + (hc * 128) // HC2
                off = (hc * 128) % HC2
                return t1[c][:, off:off + 128]

            hbt = {}

            def fc1(t, hcs=range(KH)):
                tok = slice(t * TN, (t + 1) * TN)
                for hc in hcs:
                    ps = psp.tile([128, TN], FP32, tag="hps")
                    order = [(hc + j) % KD for j in range(KD)]
                    for j, kd in enumerate(order):
                        nc.tensor.matmul(ps, t1_slice(hc, kd), xb[kd][:, tok],
                                         start=(j == 0), stop=(j == KD - 1))
                    hbv = hbp.tile([128, TN], BF16, tag="hb")
                    nc.scalar.activation(hbv, ps, Act.Gelu,
                                         bias=b1sb[:, hc:hc + 1],
                                         scale=g1h[:, 0:1])
                    hbt.setdefault(t, []).append(hbv)

            def fc2(t, t2s, g2h):
                tok = slice(t * TN, (t + 1) * TN)
                for dc in range(KD):
                    ps2 = ps2p.tile([128, TN], FP32, tag="yps")
                    order = [(dc + j) % KH for j in range(KH)]
                    for j, kh in enumerate(order):
                        lhsT = t2s[kh // 4][:, kh % 4,
                                            dc * 128:(dc + 1) * 128]
                        nc.tensor.matmul(ps2, lhsT, hbt[t][kh],
                                         start=(j == 0), stop=(j == KH - 1))
                    ysb = ysbp.tile([128, TN], FP32, tag="ysb")
                    nc.vector.tensor_scalar(
                        out=ysb, in0=ps2, scalar1=g2h[:, 0:1],
                        scalar2=b2sb[:, dc:dc + 1],
                        op0=Alu.mult, op1=Alu.add)
                    nc.sync.dma_start(out=yt[dc * 128:(dc + 1) * 128, tok],
                                      in_=ysb)
                del hbt[t]

            # ---- fc1(t0) chases the w1 quant ----
            fc1(0)

            # ---- w2 pass 1: 6 batches [128,3072] (4 kh each), gated ----
            acc2 = singles.tile([128, KH], FP32, tag="acc2")
            w2p1_dmas = []
            w2p1_tiles = []
            for bt in range(W2B):
                wf = wf2p.tile([128, 4, D], FP32, tag="w2")
                src = wt2[bt * 512:(bt + 1) * 512, :]
                dma = nc.sync.dma_start(
                    out=wf, in_=src.rearrange("(c p) f -> p c f", p=128))
                add_dep_helper(dma.ins, xb_dmas[-1].ins,
                               reason="dma order: w2p1 after x")
                w2p1_dmas.append(dma)
                w2p1_tiles.append(wf)
                for c in range(4):
                    nc.vector.tensor_reduce(
                        out=acc2[:, 4 * bt + c:4 * bt + c + 1],
                        in_=wf[:, c, :], axis=AxX, op=Alu.add,
                        apply_absolute_value=True)
            g2h, g2n = gamma_chain(acc2, KH, D * H, "g2")

            # ---- w2 pass 2 (re-read) + quant -> {-2,0,2}, scale g2/2 ----
            t2 = [None] * W2B

            def w2_quant_batch(bt, on_act):
                if bt >= W2B - 3:
                    wf = w2p1_tiles[bt]        # still resident in the ring
                else:
                    wf = wf2p.tile([128, 4, D], FP32, tag="w2")
                    src = wt2[bt * 512:(bt + 1) * 512, :]
                    dma = nc.sync.dma_start(
                        out=wf, in_=src.rearrange("(c p) f -> p c f", p=128))
                    add_dep_helper(dma.ins, w2p1_dmas[-1].ins,
                                   reason="dma order: w2p2 after w2p1")
                t = wres.tile([128, 4, D], FP8, tag="w1")
                if on_act:
                    quant_act(wf, t, g2h, g2n)
                else:
                    quant_dve(wf, t, g2h, g2n)
                t2[bt] = t

            # resident batches first (ready at gamma2), re-reads after;
            # ACT-path ones interleaved between fc1(1) GELU emission
            fc1(1, range(0, 8))
            w2_quant_batch(3, True)
            w2_quant_batch(4, False)
            w2_quant_batch(5, False)
            fc1(1, range(8, 16))
            w2_quant_batch(0, True)
            fc1(1, range(16, 24))
            w2_quant_batch(1, False)
            w2_quant_batch(2, False)

            fc2(0, t2, g2h)
            fc1(2)
            fc2(1, t2, g2h)
            fc1(3)
            fc2(2, t2, g2h)
            fc2(3, t2, g2h)

    nc.compile()
    return nc


_NC = None


def _get_nc():
    global _NC
    if _NC is None:
        _NC = build()
    return _NC


def kernel(x, w1, b1, w2, b2, _trace=False, _trace_kwargs=None):
    nc = _get_nc()
    x2 = np.ascontiguousarray(x.reshape(TOK, D).T)      # [768, 12608]
    wt1 = np.ascontiguousarray(w1.T)                    # [768, 3072]
    wt2 = np.ascontiguousarray(w2.T)                    # [3072, 768]
    b1r = np.ascontiguousarray(b1.reshape(KH, 128).T)   # [128, 24]
    b2r = np.ascontiguousarray(b2.reshape(KD, 128).T)   # [128, 6]
    in_maps = []
    for c in range(N_CORES):
        in_maps.append({
            "xt": np.ascontiguousarray(x2[:, c * TOK_PER:(c + 1) * TOK_PER]),
            "wt1": wt1, "wt2": wt2, "b1r": b1r, "b2r": b2r,
        })
    out = run_bass_kernel_spmd(nc, in_maps, list(range(N_CORES)),
                               trace=_trace, **(_trace_kwargs or {}))
    res = out.results
    yt = np.concatenate([res[c]["yt"] for c in range(N_CORES)], axis=1)
    y = np.ascontiguousarray(yt.T).reshape(B, S, D).astype(np.float32)
    if _trace:
        return y, out
    return y
